# revision 7
# baseline (speedup 1.0000x reference)
"""Distributed multi-head attention block for 8 Trainium2 NeuronCores.

Problem: y = out_proj(softmax(Q K^T / sqrt(dk)) V) for
x [4, 2048, 2048], 16 heads, dk=128, torch-Linear weights (y = x @ W^T).

Sharding: 2-way data parallel over batch pairs x 4-way tensor parallel over
head groups (4 heads / 512 features per group). Core c handles batches
[2p, 2p+1] (p = c // 4) and heads [4g .. 4g+3] (g = c % 4). Each core
computes a partial output y_c = Ot_g^T @ woT_g for its head group; the host
sums the 4 group partials per batch pair.

Layout strategy (all matmuls contract over the SBUF partition dim):
  - host pre-transposes x -> xT [d, s] and weights -> wT [d, e], so no
    on-device transposes are needed for the projections.
  - Q, K are produced head-transposed (Qt/Kt [dk, s]); V is produced natural
    ([s, dk]) by swapping the matmul operand roles.
  - scores are computed transposed, S^T [k, q] = Kt^T-slice . Qt, so the
    PV matmul (out^T [dk,q] = V^T P^T) consumes exp(S^T) directly with no
    transposes anywhere.
  - softmax denominators come from an all-ones stationary matmul
    (D [128, q] += 1s^T . P^T), every row identical, so the reciprocal
    broadcast is free. 1/sqrt(dk) is folded into wq on the host; softmax-max
    subtraction is skipped (scores ~ N(0,1), exp is safe in fp32).
  - all matmuls run as float32r (TF32-like fast path, 1 cycle/row for
    N >= 256, 4x faster than fp32) accumulating in fp32 PSUM.
"""

import sys

if "/opt/trn_rl_repo" not in sys.path:
    sys.path.insert(0, "/opt/trn_rl_repo")

import numpy as np

import concourse.bacc as bacc
import concourse.mybir as mybir
import concourse.tile as tile
from concourse.bass_utils import run_bass_kernel_spmd

F32 = mybir.dt.float32
F32R = mybir.dt.float32r

B = 4  # batch
S = 2048  # sequence length
D = 2048  # model dim
H = 16  # heads
DK = 128  # head dim

NCORES = 8
BPC = 2  # batches per core (data parallel over pairs)
GROUPS = 4  # head groups (tensor parallel)
HPG = H // GROUPS  # heads per group = 4
EG = HPG * DK  # per-group projection width = 512
SL = BPC * S  # local sequence rows per core = 4096

SB = 512  # s-block width for the projection phase
QB = 512  # q-block width for the attention phase
FB = 512  # f-block width for the output projection

_CACHE: dict = {}


def _build(nrep: int = 1):
    """Build the per-core Bass program (identical on all 8 cores)."""
    nc = bacc.Bacc("TRN2", target_bir_lowering=False, debug=False, num_devices=NCORES)

    xT = nc.dram_tensor("xT", [D, SL], F32R, kind="ExternalInput").ap()
    wqT = nc.dram_tensor("wqT", [D, EG], F32R, kind="ExternalInput").ap()
    wkT = nc.dram_tensor("wkT", [D, EG], F32R, kind="ExternalInput").ap()
    wvT = nc.dram_tensor("wvT", [D, EG], F32R, kind="ExternalInput").ap()
    woT = nc.dram_tensor("woT", [EG, D], F32R, kind="ExternalInput").ap()
    y = nc.dram_tensor("y", [SL, D], F32, kind="ExternalOutput").ap()

    DC = D // 128  # contraction chunks for the projections = 16
    KC = S // 128  # k chunks per (b, h) attention = 16

    with tile.TileContext(nc) as tc:
        with tc.tile_pool(name="dram", bufs=1, space="DRAM") as dram:
            qt = dram.tile([EG, SL], F32R)  # Qt (head-transposed, pre-scaled)
            kt = dram.tile([EG, SL], F32R)  # Kt (head-transposed)
            vv = dram.tile([SL, EG], F32R)  # V (natural)

            for _ in range(nrep):
                _emit_projections(nc, tc, xT, wqT, wkT, wvT, qt, kt, vv, DC)
                _emit_attention(nc, tc, woT, y, qt, kt, vv, KC)

    nc.compile()
    return nc


def _emit_projections(nc, tc, xT, wqT, wkT, wvT, qt, kt, vv, DC):
    """Phase 1: Qt/Kt [EG, SL] and V [SL, EG] from xT, spilled to DRAM."""
    with (
        tc.tile_pool(name="wproj", bufs=1) as wpool,
        tc.tile_pool(name="xin", bufs=2) as xpool,
        tc.tile_pool(name="pevict", bufs=4) as epool,
        tc.tile_pool(name="pproj", bufs=8, space="PSUM") as ppool,
    ):
        wq_s = wpool.tile([128, DC, EG], F32R, tag="wq")
        wk_s = wpool.tile([128, DC, EG], F32R, tag="wk")
        wv_s = wpool.tile([128, DC, EG], F32R, tag="wv")
        nc.sync.dma_start(wq_s[:], wqT.rearrange("(dc p) e -> p dc e", p=128))
        nc.sync.dma_start(wk_s[:], wkT.rearrange("(dc p) e -> p dc e", p=128))
        nc.sync.dma_start(wv_s[:], wvT.rearrange("(dc p) e -> p dc e", p=128))

        for sb in range(SL // SB):
            xts = xpool.tile([128, DC, SB], F32R, tag="xts")
            nc.sync.dma_start(
                xts[:],
                xT[:, sb * SB : (sb + 1) * SB].rearrange("(dc p) s -> p dc s", p=128),
            )
            # Qt / Kt: out[e-chunk 128, s 512] accumulated over d
            for w_s, dst in ((wq_s, qt), (wk_s, kt)):
                for ec in range(EG // 128):
                    ps = ppool.tile([128, SB], F32, tag="pp")
                    for dc in range(DC):
                        nc.tensor.matmul(
                            ps[:],
                            w_s[:, dc, ec * 128 : (ec + 1) * 128],
                            xts[:, dc, :],
                            start=(dc == 0),
                            stop=(dc == DC - 1),
                        )
                    ev = epool.tile([128, SB], F32R, tag="ev")
                    nc.vector.tensor_copy(out=ev[:], in_=ps[:])
                    nc.sync.dma_start(
                        dst[ec * 128 : (ec + 1) * 128, sb * SB : (sb + 1) * SB], ev[:]
                    )
            # V: out[s-chunk 128, e 512] accumulated over d (roles swapped)
            for sc in range(SB // 128):
                ps = ppool.tile([128, EG], F32, tag="pp")
                for dc in range(DC):
                    nc.tensor.matmul(
                        ps[:],
                        xts[:, dc, sc * 128 : (sc + 1) * 128],
                        wv_s[:, dc, :],
                        start=(dc == 0),
                        stop=(dc == DC - 1),
                    )
                ev = epool.tile([128, EG], F32R, tag="ev")
                nc.vector.tensor_copy(out=ev[:], in_=ps[:])
                r0 = sb * SB + sc * 128
                nc.sync.dma_start(vv[r0 : r0 + 128, :], ev[:])


def _emit_attention(nc, tc, woT, y, qt, kt, vv, KC, denom=True):
    """Phase 2: per (b, h) flash-style attention + per-b output projection."""
    with (
        tc.tile_pool(name="watt", bufs=1) as wpool,
        tc.tile_pool(name="qkv", bufs=2) as qkvpool,
        tc.tile_pool(name="ptile", bufs=4) as ptpool,
        tc.tile_pool(name="rdt", bufs=2) as rdpool,
        tc.tile_pool(name="ott", bufs=2 * HPG) as otpool,
        tc.tile_pool(name="yev", bufs=4) as ypool,
        tc.tile_pool(name="psatt", bufs=3, space="PSUM") as pspool,
        tc.tile_pool(name="psacc", bufs=3, space="PSUM") as popool,
        tc.tile_pool(name="psden", bufs=2, space="PSUM") as pdpool,
    ):
        wo_s = wpool.tile([128, HPG, D], F32R, tag="wo")
        nc.sync.dma_start(wo_s[:], woT.rearrange("(hc p) f -> p hc f", p=128))
        ones_f = wpool.tile([128, 128], F32, tag="ones_f")
        nc.vector.memset(ones_f[:], 1.0)
        ones = wpool.tile([128, 128], F32R, tag="ones")
        nc.vector.tensor_copy(out=ones[:], in_=ones_f[:])

        for b in range(BPC):
            s0 = b * S
            ot_tiles = []
            for h in range(HPG):
                qt_s = qkvpool.tile([128, S], F32R, tag="qts")
                kt_s = qkvpool.tile([128, S], F32R, tag="kts")
                v_s = qkvpool.tile([128, KC, DK], F32R, tag="vs")
                e0 = h * 128
                nc.sync.dma_start(qt_s[:], qt[e0 : e0 + 128, s0 : s0 + S])
                nc.sync.dma_start(kt_s[:], kt[e0 : e0 + 128, s0 : s0 + S])
                nc.sync.dma_start(
                    v_s[:],
                    vv[s0 : s0 + S, e0 : e0 + 128].rearrange(
                        "(kc p) e -> p kc e", p=128
                    ),
                )
                ot = otpool.tile([128, S], F32R, tag="ot")
                # process q-blocks in pairs: each stationary operand (Kt
                # chunk, V chunk, ones) is loaded once per PAIR of matmuls,
                # halving the weight-load traffic into the PE array.
                for qp in range(S // (2 * QB)):
                    qbs = (2 * qp, 2 * qp + 1)
                    ps_o = [popool.tile([128, QB], F32, tag="po", name=f"ps_o{i}") for i in range(2)]
                    ps_d = [pdpool.tile([128, QB], F32, tag="pd", name=f"ps_d{i}") for i in range(2)]
                    # software-pipelined: S^T tiles + exp for chunk kc+1 are
                    # emitted before the PV/denominator matmuls of chunk kc,
                    # so PE never stalls on the ACT exp.
                    pts = [None] * KC

                    def score_exp(kc, qbs=qbs, qt_s=qt_s, kt_s=kt_s, pts=pts):
                        pair = []
                        for qb in qbs:
                            ps_s = pspool.tile([128, QB], F32, tag="ps")
                            nc.tensor.matmul(
                                ps_s[:],
                                kt_s[:, kc * 128 : (kc + 1) * 128],
                                qt_s[:, qb * QB : (qb + 1) * QB],
                                start=True,
                                stop=True,
                            )
                            pt = ptpool.tile([128, QB], F32R, tag="pt")
                            nc.scalar.activation(
                                pt[:], ps_s[:], mybir.ActivationFunctionType.Exp
                            )
                            pair.append(pt)
                        pts[kc] = pair

                    score_exp(0)
                    for kc in range(KC):
                        if kc + 1 < KC:
                            score_exp(kc + 1)
                        pair = pts[kc]
                        for i in range(2):
                            nc.tensor.matmul(
                                ps_o[i][:],
                                v_s[:, kc, :],
                                pair[i][:],
                                start=(kc == 0),
                                stop=(kc == KC - 1),
                            )
                        if denom:
                            for i in range(2):
                                nc.tensor.matmul(
                                    ps_d[i][:],
                                    ones[:],
                                    pair[i][:],
                                    start=(kc == 0),
                                    stop=(kc == KC - 1),
                                )
                    for i, qb in enumerate(qbs):
                        rd = rdpool.tile([128, QB], F32, tag="rd")
                        nc.vector.reciprocal(rd[:], ps_d[i][:] if denom else ps_o[i][:])
                        nc.vector.tensor_mul(
                            ot[:, qb * QB : (qb + 1) * QB], ps_o[i][:], rd[:]
                        )
                ot_tiles.append(ot)
            # output projection for batch b: y[s, f] += Ot_h^T . woT_h
            for sc in range(S // 128):
                for fb in range(D // FB):
                    ps_y = pspool.tile([128, FB], F32, tag="ps")
                    for h in range(HPG):
                        nc.tensor.matmul(
                            ps_y[:],
                            ot_tiles[h][:, sc * 128 : (sc + 1) * 128],
                            wo_s[:, h, fb * FB : (fb + 1) * FB],
                            start=(h == 0),
                            stop=(h == HPG - 1),
                        )
                    yt = ypool.tile([128, FB], F32, tag="yt")
                    nc.vector.tensor_copy(out=yt[:], in_=ps_y[:])
                    nc.sync.dma_start(
                        y[s0 + sc * 128 : s0 + (sc + 1) * 128, fb * FB : (fb + 1) * FB],
                        yt[:],
                    )


def _prepare_in_maps(x, wq, wk, wv, wo):
    x = np.ascontiguousarray(np.asarray(x, dtype=np.float32))
    wq = np.asarray(wq, dtype=np.float32)
    wk = np.asarray(wk, dtype=np.float32)
    wv = np.asarray(wv, dtype=np.float32)
    wo = np.asarray(wo, dtype=np.float32)

    scale = np.float32(1.0 / np.sqrt(DK))
    xT_pair = [
        np.ascontiguousarray(x[2 * p : 2 * p + 2].reshape(BPC * S, D).T)
        for p in range(NCORES // GROUPS)
    ]
    wqT_g, wkT_g, wvT_g, woT_g = [], [], [], []
    for g in range(GROUPS):
        eg = slice(g * EG, (g + 1) * EG)
        wqT_g.append(np.ascontiguousarray(wq[eg].T * scale))
        wkT_g.append(np.ascontiguousarray(wk[eg].T))
        wvT_g.append(np.ascontiguousarray(wv[eg].T))
        woT_g.append(np.ascontiguousarray(wo[:, eg].T))

    in_maps = []
    for c in range(NCORES):
        p, g = c // GROUPS, c % GROUPS
        in_maps.append(
            {
                "xT": xT_pair[p],
                "wqT": wqT_g[g],
                "wkT": wkT_g[g],
                "wvT": wvT_g[g],
                "woT": woT_g[g],
            }
        )
    return in_maps


def kernel(x, wq, wk, wv, wo):
    in_maps = _prepare_in_maps(x, wq, wk, wv, wo)

    if "nc" not in _CACHE:
        _CACHE["nc"] = _build()
    nc = _CACHE["nc"]

    res = run_bass_kernel_spmd(nc, in_maps, core_ids=list(range(NCORES)))

    # host gather: sum the 4 head-group partials per batch pair
    out = np.zeros((B, S, D), dtype=np.float32)
    for p in range(NCORES // GROUPS):
        acc = res.results[p * GROUPS]["y"].copy()
        for g in range(1, GROUPS):
            acc += res.results[p * GROUPS + g]["y"]
        out[2 * p : 2 * p + 2] = acc.reshape(BPC, S, D)
    return out
